# revision 42
# baseline (speedup 1.0000x reference)
"""Trainium2 Bass kernel for nn_Aggregation (sparse local attention aggregation).

out[n, g*64+cw, y, x] = sum_{i,j in 3x3} input[n, g*64+cw, y+i-1, x+j-1]
                        * weight[n, cw, i*3+j, y*64+x]

Sharding: data-parallel over batch n: 8 cores x 2 batches each.

Per-core layouts (host pre-swizzled, fp16 on the wire; every DMA is a
[128 partitions x contiguous] transfer):
  x_t  : [128=(b,cw), H, (g, 66)] column-padded [0, x0..x63, 0] per group:
         the dj=-1 / dj=+1 shifted reads start at even element offsets so
         DVE tensor_tensor stays in 2x packed mode.
  xf_t : [128=(b,cw), H, (g, 64)] flat unpadded copy for the dj=0 taps
         (the padded layout would 2-byte-misalign them -> 1x mode).
  w_t  : [128=(b,cw), H, (di, [dj-1, dj+1, dj0], x)] - taps reordered so
         each (di, dj=+-1) pair is a contiguous 128-wide block.
  o_t  : [128=(b,cw), H, (g, x)]  fp16, host upcasts to f32.

Engine split per chunk of output rows (trace-derived):
  DVE : binding engine (~154us/core of pure 2x-mode tensor_tensor work).
        6 instructions per chunk: 3 "pair" TTs (dj=-1 and dj=+1 for one
        di, reading overlapping windows of the padded x via a custom
        merged AP [rg, dj, c]) + 3 dj=0 TTs from the flat copy. The ISA
        caps TENSOR_TENSOR APs at 3 free dims, which this just fits.
        Fusing further (one TT for all dj=0 planes) was measured SLOWER:
        the saved dispatch time is hidden by overlap anyway, and the
        coarser PE consumption granularity backloads the pipeline tail.
  PE  : 9 identity-matmul accumulate passes into PSUM (per 512-col bank).
  ACT : PSUM->SBUF evacuation with f32->fp16 cast.
  DMA : w/x/xf chunk loads (w first: its completion gates the chunk's
        first TT), out store. Transfers are FIFO per HWDGE ring, so the
        chunk sizes ramp 2,4,6,8... to keep the head of the pipeline
        from outrunning the DMA stream.
"""

import numpy as np

N, C, H, W = 16, 512, 64, 64
CW, G, K = 64, 8, 3
NCORE = 8
NB = N // NCORE          # batches per core

R = 8                    # max chunk rows
RP = R + 2               # max plane rows incl. halo
WP = W + 2               # 66
GW = G * W               # 512
GWP = G * WP             # 528: one padded row-block (all groups)
WROW = K * K * W         # 576

CHUNKS = [2, 2, 4, 4, 6, 8, 8, 8, 8, 8, 4, 2]
# host-side tap reorder within each w row: (di, [djc0, djc2, djc1], x)
W_ORDER = [0, 2, 1, 3, 5, 4, 6, 8, 7]

_cache = {}


def _build():
    import concourse.mybir as mybir
    from concourse import bacc
    from concourse.ap import AP
    from concourse.tile import TileContext
    from concourse.masks import make_identity

    f16 = mybir.dt.float16
    f32 = mybir.dt.float32

    nc = bacc.Bacc()
    x_t = nc.dram_tensor("x_t", [128, H, GWP], f16, kind="ExternalInput")
    xf_t = nc.dram_tensor("xf_t", [128, H, GW], f16, kind="ExternalInput")
    w_t = nc.dram_tensor("w_t", [128, H, WROW], f16, kind="ExternalInput")
    o_t = nc.dram_tensor("o_t", [128, H, GW], f16, kind="ExternalOutput")

    PL = RP * GWP            # padded plane length per partition (max chunk)
    FL = RP * GW             # flat plane length per partition (max chunk)

    def strided(base, off, dims):
        # custom free dims on a tile's AP (keeps the partition dim);
        # allows overlapping windows that rearrange() cannot express.
        return AP(
            base.tensor, base.offset + off,
            [list(base.ap[0])] + [list(d) for d in dims],
        )

    with TileContext(nc) as tc:
        with (
            tc.tile_pool(name="const", bufs=1) as const_pool,
            tc.tile_pool(name="xe", bufs=3) as xe_pool,
            tc.tile_pool(name="xf", bufs=3) as xf_pool,
            tc.tile_pool(name="wt", bufs=3) as wt_pool,
            tc.tile_pool(name="pr", bufs=1) as pr_pool,
            tc.tile_pool(name="os", bufs=2) as os_pool,
            tc.tile_pool(name="ps", bufs=1, space="PSUM") as ps_pool,
        ):
            # Two identity copies: alternating the stationary operand lets
            # each LDWEIGHTS target the background weight buffer and overlap
            # the in-flight matmul (same-tensor LDW serializes instead).
            ident = const_pool.tile([128, 128], f16)
            make_identity(nc, ident)
            ident2 = const_pool.tile([128, 128], f16)
            make_identity(nc, ident2)
            idents = [ident, ident2]
            # Warm the ACT function table during the boot phase so the
            # one-time ACT_TABLE_LOAD (~1.3us) doesn't delay the first evac.
            warm = const_pool.tile([128, 1], f16)
            nc.scalar.copy(out=warm[:], in_=ident[:, 0:1])

            y0 = 0
            for ci, Rc in enumerate(CHUNKS):
                RPc = Rc + 2
                row_lo = max(y0 - 1, 0)             # first loaded image row
                row_hi = min(y0 + Rc + 1, H)        # one past last loaded row
                RL = row_hi - row_lo                # rows loaded
                prow0 = 0 if y0 > 0 else 1          # plane row of first loaded row

                # ---- loads; w first: the chunk's first TT waits on it and
                # the HWDGE ring drains transfers FIFO.
                wt = wt_pool.tile([128, R * WROW], f16, tag="wt")
                nc.sync.dma_start(
                    out=wt[:, : Rc * WROW], in_=w_t[:, y0 : y0 + Rc, :]
                )
                # xe/xf go through the Scalar queue: DMA transfers drain
                # FIFO per HWDGE ring, and issuing from ACT routes these to
                # the second ring (qActDynamicHW) so the x loads run in
                # parallel with w/out on the Sync ring.
                xe = xe_pool.tile([128, PL + 66], f16, tag="xe")
                nc.scalar.dma_start(
                    out=xe[:, prow0 * GWP : (prow0 + RL) * GWP],
                    in_=x_t[:, row_lo:row_hi, :],
                )
                xf = xf_pool.tile([128, FL + 64], f16, tag="xf")
                nc.scalar.dma_start(
                    out=xf[:, prow0 * GW : (prow0 + RL) * GW],
                    in_=xf_t[:, row_lo:row_hi, :],
                )
                if y0 == 0:
                    nc.gpsimd.memset(xe[:, 0:GWP], 0.0)
                    nc.gpsimd.memset(xf[:, 0:GW], 0.0)
                if y0 + Rc == H:
                    nc.gpsimd.memset(xe[:, (RPc - 1) * GWP : RPc * GWP], 0.0)
                    nc.gpsimd.memset(xf[:, (RPc - 1) * GW : RPc * GW], 0.0)

                # ---- DVE products: 3 pair TTs (dj=+-1) + 3 dj=0 TTs.
                # From chunk 3 on, production interleaves pair/single so
                # the PE accumulation gated behind the chunk's final TTs
                # shrinks from 3 planes to 1 small plane (the trailing PE
                # work otherwise cascades into a ~2.4us post-stream tail).
                # The ramp chunks keep pairs first: their dj=0 source (xf)
                # lands after xe on the act DMA ring.
                def emit_pair(di):
                    xsrc = strided(
                        xe[:], di * GWP,
                        [(WP, G * Rc), (2, 2), (1, W)],
                    )
                    wsrc = strided(
                        wt[:], di * 3 * W,
                        [(WROW, Rc), (0, G), (1, 2 * W)],
                    )
                    pp = pr_pool.tile([128, 2 * R * GW], f16, tag=f"pp{di}")
                    pdst = strided(
                        pp[:], 0,
                        [(W, G * Rc), (Rc * GW, 2), (1, W)],
                    )
                    nc.vector.tensor_mul(out=pdst, in0=xsrc, in1=wsrc)
                    return [(pp, 0), (pp, Rc * GW)]

                def emit_single(di):
                    xsrc = strided(
                        xf[:], di * GW,
                        [(GW, Rc), (W, G), (1, W)],
                    )
                    wsrc = strided(
                        wt[:], di * 3 * W + 2 * W,
                        [(WROW, Rc), (0, G), (1, W)],
                    )
                    ps_ = pr_pool.tile([128, R * GW], f16, tag=f"ps{di}")
                    pdst = strided(ps_[:], 0, [(GW, Rc), (W, G), (1, W)])
                    nc.vector.tensor_mul(out=pdst, in0=xsrc, in1=wsrc)
                    return [(ps_, 0)]

                if ci >= 3:
                    order = [("p", 0), ("s", 0), ("p", 1),
                             ("s", 1), ("p", 2), ("s", 2)]
                else:
                    order = [("p", 0), ("p", 1), ("p", 2),
                             ("s", 0), ("s", 1), ("s", 2)]
                plane_srcs = []
                for kind, di in order:
                    plane_srcs += emit_pair(di) if kind == "p" else emit_single(di)

                # ---- PE accumulate: 9 planes in production order,
                # Rc x 512-col matmuls per pass across 4-bank PSUM tiles
                os_ = os_pool.tile([128, R * GW], f16, tag="os")
                nt = (Rc + 3) // 4                  # PSUM tiles needed
                ps_tiles = [
                    ps_pool.tile(
                        [128, 4 * 512], f32, tag=f"psum{(ci + t) % 2}",
                        name=f"ps_{ci}_{t}",
                    )
                    for t in range(nt)
                ]
                mm = 0
                for pi, (tile, base) in enumerate(plane_srcs):
                    for r in range(Rc):
                        nc.tensor.matmul(
                            ps_tiles[r // 4][:, (r % 4) * 512 : (r % 4 + 1) * 512],
                            idents[mm % 2],
                            tile[:, base + r * 512 : base + (r + 1) * 512],
                            start=(pi == 0),
                            stop=(pi == len(plane_srcs) - 1),
                        )
                        mm += 1
                # ---- evacuate PSUM (f32 -> fp16) and store
                for t in range(nt):
                    hi = min(4 * 512, (Rc - 4 * t) * 512)
                    nc.scalar.copy(
                        out=os_[:, t * 2048 : t * 2048 + hi],
                        in_=ps_tiles[t][:, :hi],
                    )
                nc.sync.dma_start(
                    out=o_t[:, y0 : y0 + Rc, :], in_=os_[:, : Rc * GW]
                )
                y0 += Rc

    nc.finalize()
    return nc


def _get():
    if "nc" not in _cache:
        _cache["nc"] = _build()
    return _cache["nc"]


def _swizzle_core(inp, wgt):
    # inp [2, 512, 64, 64] -> xe [128, H, G*66] fp16 zero-padded columns
    # (p = b*64+cw, free = (y, g, 66)) plus xf [128, H, G*64] unpadded.
    a = inp.reshape(NB, G, CW, H, W).transpose(0, 2, 3, 1, 4)  # b,cw,y,g,x
    a16 = np.ascontiguousarray(a, dtype=np.float16)
    xe = np.zeros((NB, CW, H, G, WP), dtype=np.float16)
    xe[..., 1 : W + 1] = a16
    xe = xe.reshape(128, H, GWP)
    xf = a16.reshape(128, H, GW)
    # wgt [2, 64, 9, 4096] -> [128, H, 9*W] fp16; free = (y, ij', x) with
    # the taps reordered (di, [dj-1, dj+1, dj0]).
    b = wgt.reshape(NB, CW, K * K, H, W)[:, :, W_ORDER]
    b = b.transpose(0, 1, 3, 2, 4)
    wt = np.ascontiguousarray(b, dtype=np.float16).reshape(128, H, WROW)
    return xe, xf, wt


def _unswizzle_core(o):
    # [128, H, G*W] fp16 -> [2, 512, 64, 64] f32
    a = o.reshape(NB, CW, H, G, W).astype(np.float32).transpose(0, 3, 1, 2, 4)
    return np.ascontiguousarray(a).reshape(NB, C, H, W)


def kernel(input: np.ndarray, weight: np.ndarray) -> np.ndarray:
    from concourse.bass_utils import run_bass_kernel_spmd

    input = np.ascontiguousarray(input, dtype=np.float32)
    weight = np.ascontiguousarray(weight, dtype=np.float32)
    nc = _get()
    in_maps = []
    for i in range(NCORE):
        xe, xf, wt = _swizzle_core(
            input[i * NB : (i + 1) * NB], weight[i * NB : (i + 1) * NB]
        )
        in_maps.append({"x_t": xe, "xf_t": xf, "w_t": wt})
    res = run_bass_kernel_spmd(nc, in_maps, core_ids=list(range(NCORE)))
    return np.concatenate(
        [_unswizzle_core(res.results[i]["o_t"]) for i in range(NCORE)], axis=0
    )


# revision 45
# speedup vs baseline: 1.0108x; 1.0108x over previous
"""Trainium2 Bass kernel for nn_Aggregation (sparse local attention aggregation).

out[n, g*64+cw, y, x] = sum_{i,j in 3x3} input[n, g*64+cw, y+i-1, x+j-1]
                        * weight[n, cw, i*3+j, y*64+x]

Sharding: data-parallel over batch n: 8 cores x 2 batches each.

Per-core layouts (host pre-swizzled, fp16 on the wire; every DMA is a
[128 partitions x contiguous] transfer):
  x_t  : [128=(b,cw), H, (g, 66)] column-padded [0, x0..x63, 0] per group:
         the dj=-1 / dj=+1 shifted reads start at even element offsets so
         DVE tensor_tensor stays in 2x packed mode.
  xf_t : [128=(b,cw), H, (g, 64)] flat unpadded copy for the dj=0 taps
         (the padded layout would 2-byte-misalign them -> 1x mode).
  w_t  : [128=(b,cw), H, (di, [dj-1, dj+1, dj0], x)] - taps reordered so
         each (di, dj=+-1) pair is a contiguous 128-wide block.
  o_t  : [128=(b,cw), H, (g, x)]  fp16, host upcasts to f32.

Engine split per chunk of output rows (trace-derived):
  DVE : binding engine (~154us/core of pure 2x-mode tensor_tensor work).
        6 instructions per chunk: 3 "pair" TTs (dj=-1 and dj=+1 for one
        di, reading overlapping windows of the padded x via a custom
        merged AP [rg, dj, c]) + 3 dj=0 TTs from the flat copy. The ISA
        caps TENSOR_TENSOR APs at 3 free dims, which this just fits.
        Fusing further (one TT for all dj=0 planes) was measured SLOWER:
        the saved dispatch time is hidden by overlap anyway, and the
        coarser PE consumption granularity backloads the pipeline tail.
  PE  : 9 identity-matmul accumulate passes into PSUM (per 512-col bank).
  ACT : PSUM->SBUF evacuation with f32->fp16 cast.
  DMA : w/x/xf chunk loads (w first: its completion gates the chunk's
        first TT), out store. Transfers are FIFO per HWDGE ring, so the
        chunk sizes ramp 2,4,6,8... to keep the head of the pipeline
        from outrunning the DMA stream.
"""

import numpy as np

N, C, H, W = 16, 512, 64, 64
CW, G, K = 64, 8, 3
NCORE = 8
NB = N // NCORE          # batches per core

R = 8                    # max chunk rows
RP = R + 2               # max plane rows incl. halo
WP = W + 2               # 66
GW = G * W               # 512
GWP = G * WP             # 528: one padded row-block (all groups)
WROW = K * K * W         # 576

CHUNKS = [2, 2, 4, 6, 8, 8, 8, 8, 8, 8, 2]
# host-side tap reorder within each w row: (di, [djc0, djc2, djc1], x)
W_ORDER = [0, 2, 1, 3, 5, 4, 6, 8, 7]

_cache = {}


def _build():
    import concourse.mybir as mybir
    from concourse import bacc
    from concourse.ap import AP
    from concourse.tile import TileContext
    from concourse.masks import make_identity

    f16 = mybir.dt.float16
    f32 = mybir.dt.float32

    nc = bacc.Bacc()
    x_t = nc.dram_tensor("x_t", [128, H, GWP], f16, kind="ExternalInput")
    xf_t = nc.dram_tensor("xf_t", [128, H, GW], f16, kind="ExternalInput")
    w_t = nc.dram_tensor("w_t", [128, H, WROW], f16, kind="ExternalInput")
    o_t = nc.dram_tensor("o_t", [128, H, GW], f16, kind="ExternalOutput")

    PL = RP * GWP            # padded plane length per partition (max chunk)
    FL = RP * GW             # flat plane length per partition (max chunk)

    def strided(base, off, dims):
        # custom free dims on a tile's AP (keeps the partition dim);
        # allows overlapping windows that rearrange() cannot express.
        return AP(
            base.tensor, base.offset + off,
            [list(base.ap[0])] + [list(d) for d in dims],
        )

    with TileContext(nc) as tc:
        with (
            tc.tile_pool(name="const", bufs=1) as const_pool,
            tc.tile_pool(name="xe", bufs=3) as xe_pool,
            tc.tile_pool(name="xf", bufs=3) as xf_pool,
            tc.tile_pool(name="wt", bufs=3) as wt_pool,
            tc.tile_pool(name="pr", bufs=1) as pr_pool,
            tc.tile_pool(name="os", bufs=2) as os_pool,
            tc.tile_pool(name="ps", bufs=1, space="PSUM") as ps_pool,
        ):
            # Two identity copies: alternating the stationary operand lets
            # each LDWEIGHTS target the background weight buffer and overlap
            # the in-flight matmul (same-tensor LDW serializes instead).
            ident = const_pool.tile([128, 128], f16)
            make_identity(nc, ident)
            ident2 = const_pool.tile([128, 128], f16)
            make_identity(nc, ident2)
            idents = [ident, ident2]
            # Warm the ACT function table during the boot phase so the
            # one-time ACT_TABLE_LOAD (~1.3us) doesn't delay the first evac.
            warm = const_pool.tile([128, 1], f16)
            nc.scalar.copy(out=warm[:], in_=ident[:, 0:1])

            y0 = 0
            for ci, Rc in enumerate(CHUNKS):
                RPc = Rc + 2
                row_lo = max(y0 - 1, 0)             # first loaded image row
                row_hi = min(y0 + Rc + 1, H)        # one past last loaded row
                RL = row_hi - row_lo                # rows loaded
                prow0 = 0 if y0 > 0 else 1          # plane row of first loaded row

                # ---- loads; w first: the chunk's first TT waits on it and
                # the HWDGE ring drains transfers FIFO.
                wt = wt_pool.tile([128, R * WROW], f16, tag="wt")
                nc.sync.dma_start(
                    out=wt[:, : Rc * WROW], in_=w_t[:, y0 : y0 + Rc, :]
                )
                # xe/xf go through the Scalar queue: DMA transfers drain
                # FIFO per HWDGE ring, and issuing from ACT routes these to
                # the second ring (qActDynamicHW) so the x loads run in
                # parallel with w/out on the Sync ring.
                xe = xe_pool.tile([128, PL + 66], f16, tag="xe")
                nc.scalar.dma_start(
                    out=xe[:, prow0 * GWP : (prow0 + RL) * GWP],
                    in_=x_t[:, row_lo:row_hi, :],
                )
                xf = xf_pool.tile([128, FL + 64], f16, tag="xf")
                nc.scalar.dma_start(
                    out=xf[:, prow0 * GW : (prow0 + RL) * GW],
                    in_=xf_t[:, row_lo:row_hi, :],
                )
                if y0 == 0:
                    nc.gpsimd.memset(xe[:, 0:GWP], 0.0)
                    nc.gpsimd.memset(xf[:, 0:GW], 0.0)
                if y0 + Rc == H:
                    nc.gpsimd.memset(xe[:, (RPc - 1) * GWP : RPc * GWP], 0.0)
                    nc.gpsimd.memset(xf[:, (RPc - 1) * GW : RPc * GW], 0.0)

                # ---- DVE products: 3 pair TTs (dj=+-1) then 3 dj=0 TTs.
                # (Interleaving pair/single production was measured SLOWER:
                # drain unchanged, +1.5us of new mid-stream gaps - the
                # pairs-first order below is the validated optimum.)
                def emit_pair(di):
                    xsrc = strided(
                        xe[:], di * GWP,
                        [(WP, G * Rc), (2, 2), (1, W)],
                    )
                    wsrc = strided(
                        wt[:], di * 3 * W,
                        [(WROW, Rc), (0, G), (1, 2 * W)],
                    )
                    pp = pr_pool.tile([128, 2 * R * GW], f16, tag=f"pp{di}")
                    pdst = strided(
                        pp[:], 0,
                        [(W, G * Rc), (Rc * GW, 2), (1, W)],
                    )
                    nc.vector.tensor_mul(out=pdst, in0=xsrc, in1=wsrc)
                    return [(pp, 0), (pp, Rc * GW)]

                def emit_single(di):
                    xsrc = strided(
                        xf[:], di * GW,
                        [(GW, Rc), (W, G), (1, W)],
                    )
                    wsrc = strided(
                        wt[:], di * 3 * W + 2 * W,
                        [(WROW, Rc), (0, G), (1, W)],
                    )
                    ps_ = pr_pool.tile([128, R * GW], f16, tag=f"ps{di}")
                    pdst = strided(ps_[:], 0, [(GW, Rc), (W, G), (1, W)])
                    nc.vector.tensor_mul(out=pdst, in0=xsrc, in1=wsrc)
                    return [(ps_, 0)]

                order = [("p", 0), ("p", 1), ("p", 2),
                         ("s", 0), ("s", 1), ("s", 2)]
                plane_srcs = []
                for kind, di in order:
                    plane_srcs += emit_pair(di) if kind == "p" else emit_single(di)

                # ---- PE accumulate: 9 planes in production order,
                # Rc x 512-col matmuls per pass across 4-bank PSUM tiles
                os_ = os_pool.tile([128, R * GW], f16, tag="os")
                nt = (Rc + 3) // 4                  # PSUM tiles needed
                ps_tiles = [
                    ps_pool.tile(
                        [128, 4 * 512], f32, tag=f"psum{(ci + t) % 2}",
                        name=f"ps_{ci}_{t}",
                    )
                    for t in range(nt)
                ]
                mm = 0
                for pi, (tile, base) in enumerate(plane_srcs):
                    for r in range(Rc):
                        nc.tensor.matmul(
                            ps_tiles[r // 4][:, (r % 4) * 512 : (r % 4 + 1) * 512],
                            idents[mm % 2],
                            tile[:, base + r * 512 : base + (r + 1) * 512],
                            start=(pi == 0),
                            stop=(pi == len(plane_srcs) - 1),
                        )
                        mm += 1
                # ---- evacuate PSUM (f32 -> fp16) and store
                for t in range(nt):
                    hi = min(4 * 512, (Rc - 4 * t) * 512)
                    nc.scalar.copy(
                        out=os_[:, t * 2048 : t * 2048 + hi],
                        in_=ps_tiles[t][:, :hi],
                    )
                nc.sync.dma_start(
                    out=o_t[:, y0 : y0 + Rc, :], in_=os_[:, : Rc * GW]
                )
                y0 += Rc

    nc.finalize()
    return nc


def _get():
    if "nc" not in _cache:
        _cache["nc"] = _build()
    return _cache["nc"]


def _swizzle_core(inp, wgt):
    # inp [2, 512, 64, 64] -> xe [128, H, G*66] fp16 zero-padded columns
    # (p = b*64+cw, free = (y, g, 66)) plus xf [128, H, G*64] unpadded.
    a = inp.reshape(NB, G, CW, H, W).transpose(0, 2, 3, 1, 4)  # b,cw,y,g,x
    a16 = np.ascontiguousarray(a, dtype=np.float16)
    xe = np.zeros((NB, CW, H, G, WP), dtype=np.float16)
    xe[..., 1 : W + 1] = a16
    xe = xe.reshape(128, H, GWP)
    xf = a16.reshape(128, H, GW)
    # wgt [2, 64, 9, 4096] -> [128, H, 9*W] fp16; free = (y, ij', x) with
    # the taps reordered (di, [dj-1, dj+1, dj0]).
    b = wgt.reshape(NB, CW, K * K, H, W)[:, :, W_ORDER]
    b = b.transpose(0, 1, 3, 2, 4)
    wt = np.ascontiguousarray(b, dtype=np.float16).reshape(128, H, WROW)
    return xe, xf, wt


def _unswizzle_core(o):
    # [128, H, G*W] fp16 -> [2, 512, 64, 64] f32
    a = o.reshape(NB, CW, H, G, W).astype(np.float32).transpose(0, 3, 1, 2, 4)
    return np.ascontiguousarray(a).reshape(NB, C, H, W)


def kernel(input: np.ndarray, weight: np.ndarray) -> np.ndarray:
    from concourse.bass_utils import run_bass_kernel_spmd

    input = np.ascontiguousarray(input, dtype=np.float32)
    weight = np.ascontiguousarray(weight, dtype=np.float32)
    nc = _get()
    in_maps = []
    for i in range(NCORE):
        xe, xf, wt = _swizzle_core(
            input[i * NB : (i + 1) * NB], weight[i * NB : (i + 1) * NB]
        )
        in_maps.append({"x_t": xe, "xf_t": xf, "w_t": wt})
    res = run_bass_kernel_spmd(nc, in_maps, core_ids=list(range(NCORE)))
    return np.concatenate(
        [_unswizzle_core(res.results[i]["o_t"]) for i in range(NCORE)], axis=0
    )
